# revision 1
# baseline (speedup 1.0000x reference)
"""Self-contained 8-core Trainium2 kernel for the 6-layer dense transformer.

Sharding: token-parallel. Core c owns batch b=c//2, sequence half h=c%2
(512 tokens). Per layer, each core computes K/V for its own tokens and
AllGathers them; causal attention then runs locally over the (batch's) full
key prefix, selected from the AG buffer with host-computed indirect-DMA
indices. The LM head is vocab-sharded: after a final AllGather of the
normalized activations, each core computes logits for all 4096 tokens over
its 4000-column vocab slice.

Activations live feature-major (x^T: [D, tokens]) so every matmul contracts
over the partition axis without transposes. Matmuls run in float32r
(TF32-like, ~1e-4 relative error, full PE rate); LayerNorm statistics use
matmul-with-ones partition reductions; softmax runs on transposed scores
with the denominator produced by a ones-column appended to V.
"""

import numpy as np

B, T, D, H, HS, L, DFF, V = 4, 1024, 1024, 16, 64, 6, 4096, 32000
NCORES = 8
TC = 512            # tokens per core
P = 128
VS = V // NCORES    # 4000 vocab cols per core
EPS = 1e-5
NEG = -30000.0
QS = [0, 128, 256, 256]   # q-slice starts for key-chunks 4..7 (capped at 256)

_CACHE = {}
TRACE = False
LAST_RESULTS = None
SKIP_COLL = False   # timing variant: replace AllGathers with local 2MB DMA copies
SKIP_LM = False     # timing variant: skip the LM head


def _build():
    import concourse.bacc as bacc
    import concourse.tile as tile
    import concourse.mybir as mybir
    import concourse.bass as bass
    from concourse.masks import make_identity
    from contextlib import ExitStack

    f32 = mybir.dt.float32
    f32r = mybir.dt.float32r
    bf16 = mybir.dt.bfloat16
    i32 = mybir.dt.int32
    AF = mybir.ActivationFunctionType
    ALU = mybir.AluOpType

    nc = bacc.Bacc(None, target_bir_lowering=False, debug=False,
                   num_devices=NCORES)

    # ---- parameters ----
    tokidx = nc.declare_dram_parameter("tokidx", [TC, 1], i32, isOutput=False)
    pos = nc.declare_dram_parameter("pos", [TC, D], f32, isOutput=False)
    tokemb = nc.declare_dram_parameter("tokemb", [V, D], f32, isOutput=False)
    wqT = nc.declare_dram_parameter("wqT", [L, D, D], f32r, isOutput=False)
    wkT = nc.declare_dram_parameter("wkT", [L, D, D], f32r, isOutput=False)
    wvT = nc.declare_dram_parameter("wvT", [L, D, D], f32r, isOutput=False)
    wo = nc.declare_dram_parameter("wo", [L, D, D], f32r, isOutput=False)
    w1 = nc.declare_dram_parameter("w1", [L, D, DFF], f32r, isOutput=False)
    w2 = nc.declare_dram_parameter("w2", [L, DFF, D], f32r, isOutput=False)
    wout = nc.declare_dram_parameter("wout", [D, VS], f32r, isOutput=False)
    ln1g = nc.declare_dram_parameter("ln1g", [L, D], f32, isOutput=False)
    ln1b = nc.declare_dram_parameter("ln1b", [L, D], f32, isOutput=False)
    ln2g = nc.declare_dram_parameter("ln2g", [L, D], f32, isOutput=False)
    ln2b = nc.declare_dram_parameter("ln2b", [L, D], f32, isOutput=False)
    lnfg = nc.declare_dram_parameter("lnfg", [1, D], f32, isOutput=False)
    lnfb = nc.declare_dram_parameter("lnfb", [1, D], f32, isOutput=False)
    bo_p = nc.declare_dram_parameter("bo", [L, D], f32, isOutput=False)
    b1_p = nc.declare_dram_parameter("b1", [L, DFF], f32, isOutput=False)
    b2_p = nc.declare_dram_parameter("b2", [L, D], f32, isOutput=False)
    bout = nc.declare_dram_parameter("bout", [1, VS], f32r, isOutput=False)
    mask0 = nc.declare_dram_parameter("mask0", [4, P, TC], bf16, isOutput=False)
    mask1 = nc.declare_dram_parameter("mask1", [4, P, TC], bf16, isOutput=False)
    kidx = nc.declare_dram_parameter("kidx", [P, H], i32, isOutput=False)
    vidx = nc.declare_dram_parameter("vidx", [P, 32], i32, isOutput=False)
    out = nc.declare_dram_parameter("out", [B * T, VS], f32, isOutput=True)

    RG = [list(range(NCORES))]

    with tile.TileContext(nc) as tc:
        outer = ExitStack()
        singles = outer.enter_context(tc.tile_pool(name="singles", bufs=1))
        dramp = outer.enter_context(tc.tile_pool(name="dramp", bufs=1, space="DRAM"))

        # ---- internal DRAM ----
        k_loc = dramp.tile([D, TC], f32r, name="k_loc")
        v_loc = dramp.tile([4 * TC, 260], f32r, name="v_loc")
        xf_loc = dramp.tile([D, TC], f32r, name="xf_loc")
        k_ags = [dramp.tile([NCORES * D, TC], f32r, name=f"k_ag_{i}",
                            addr_space="Shared") for i in range(L)]
        v_ags = [dramp.tile([NCORES * 4 * TC, 260], f32r, name=f"v_ag_{i}",
                            addr_space="Shared") for i in range(L)]
        xf_ag = dramp.tile([NCORES * D, TC], f32r, name="xf_ag", addr_space="Shared")

        # constants
        ones_f = singles.tile([P, 144], f32, name="ones_f")
        nc.vector.memset(ones_f[:], 1.0)
        ones_r = singles.tile([P, 144], f32r, name="ones_r")
        nc.vector.tensor_copy(out=ones_r[:], in_=ones_f[:])
        eps_t = singles.tile([1, 1], f32, name="eps_t")
        nc.vector.memset(eps_t[:], EPS)
        ident = singles.tile([P, P], f32, name="ident")
        make_identity(nc, ident[:])
        kidx_t = singles.tile([P, H], i32, name="kidx_t")
        nc.sync.dma_start(out=kidx_t[:], in_=kidx[:])
        vidx_t = singles.tile([P, 32], i32, name="vidx_t")
        nc.sync.dma_start(out=vidx_t[:], in_=vidx[:])
        m0_t = []
        m1_t = []
        for j in range(4):
            mt = singles.tile([P, TC], bf16, name=f"m0_{j}")
            nc.sync.dma_start(out=mt[:], in_=mask0[j])
            m0_t.append(mt)
            mt = singles.tile([P, TC], bf16, name=f"m1_{j}")
            nc.sync.dma_start(out=mt[:], in_=mask1[j])
            m1_t.append(mt)

        est = ExitStack()
        lp = est.enter_context(tc.tile_pool(name="lp", bufs=1))      # xr/xln
        big = est.enter_context(tc.tile_pool(name="big", bufs=1))    # 16 shared slots
        wA = est.enter_context(tc.tile_pool(name="wA", bufs=2))      # [P,8,128] weights
        wB = est.enter_context(tc.tile_pool(name="wB", bufs=2))      # [P,8,256] wv quarters
        sp = est.enter_context(tc.tile_pool(name="sp", bufs=2))      # stream tiles
        kvp = est.enter_context(tc.tile_pool(name="kvp", bufs=2))    # kv copyback
        ktp = est.enter_context(tc.tile_pool(name="ktp", bufs=2))    # K gathers
        esp = est.enter_context(tc.tile_pool(name="esp", bufs=3))    # exp(scores)
        vtp = est.enter_context(tc.tile_pool(name="vtp", bufs=1))    # V gathers (8 tags)
        otp = est.enter_context(tc.tile_pool(name="otp", bufs=2))    # o tmp / recip
        stp = est.enter_context(tc.tile_pool(name="stp", bufs=1))    # LN stats [1,*]

        ps_mm = est.enter_context(tc.tile_pool(name="ps_mm", bufs=2, space="PSUM"))
        ps_o = est.enter_context(tc.tile_pool(name="ps_o", bufs=2, space="PSUM"))
        ps_st = est.enter_context(tc.tile_pool(name="ps_st", bufs=1, space="PSUM"))
        ps_bc = est.enter_context(tc.tile_pool(name="ps_bc", bufs=1, space="PSUM"))

        def mmtile():
            return ps_mm.tile([P, TC], f32, name="mm", tag="mm")

        xr = [lp.tile([P, TC], f32, name=f"xr_{j}", tag=f"xr_{j}") for j in range(8)]

        def xln_tiles():
            return [lp.tile([P, TC], f32r, name=f"xln_{j}", tag=f"xln_{j}")
                    for j in range(8)]

        def big_tile(i, name, dtype=f32r):
            return big.tile([P, TC], dtype, name=name, tag=f"big_{i}")

        # ---- embedding: gather + pos add + transpose into xr ----
        with tc.tile_pool(name="embp", bufs=1) as embp:
            for t4 in range(4):
                it = embp.tile([P, 1], i32, name="emb_idx", tag="emb_idx")
                nc.sync.dma_start(out=it[:], in_=tokidx[t4 * P:(t4 + 1) * P, :])
                gx = embp.tile([P, D], f32, name="emb_gx", tag="emb_gx")
                nc.gpsimd.indirect_dma_start(
                    out=gx[:], out_offset=None, in_=tokemb[:],
                    in_offset=bass.IndirectOffsetOnAxis(ap=it[:, :1], axis=0))
                pt = embp.tile([P, D], f32, name="emb_pos", tag="emb_pos")
                nc.sync.dma_start(out=pt[:], in_=pos[t4 * P:(t4 + 1) * P, :])
                xs = embp.tile([P, D], f32, name="emb_xs", tag="emb_xs")
                nc.vector.tensor_add(out=xs[:], in0=gx[:], in1=pt[:])
                for j in range(8):
                    tp = mmtile()
                    nc.tensor.transpose(out=tp[:, 0:P], in_=xs[:, j * P:(j + 1) * P],
                                        identity=ident[:])
                    nc.scalar.activation(out=xr[j][:, t4 * P:(t4 + 1) * P],
                                         in_=tp[:, 0:P], func=AF.Copy)

        # ---- ones columns of v_loc (V writes never touch them) ----
        vl_view = v_loc[:].rearrange("(hq t) (h c) -> hq t h c", hq=4, c=65)
        for hq in range(4):
            for t4 in range(4):
                nc.sync.dma_start(
                    out=vl_view[hq, t4 * P:(t4 + 1) * P, 0:4, 64:65],
                    in_=ones_r[:, 128:132])

        def layer_norm(g_t, gcol, b_t, bcol, out_tiles):
            """xr (f32) -> out_tiles (f32r); feature-major LN over partitions."""
            sum_ps = ps_st.tile([1, TC], f32, name="sum_ps", tag="st_a")
            sumsq_ps = ps_st.tile([1, TC], f32, name="sumsq_ps", tag="st_b")
            for j in range(8):
                xc = sp.tile([P, TC], f32r, name="ln_xc", tag="ln_xc")
                nc.scalar.activation(out=xc[:], in_=xr[j][:], func=AF.Copy)
                sq = sp.tile([P, TC], f32r, name="ln_sq", tag="ln_sq")
                nc.scalar.activation(out=sq[:], in_=xr[j][:], func=AF.Square)
                nc.tensor.matmul(out=sum_ps[:], lhsT=ones_r[:, 0:1], rhs=xc[:],
                                 start=(j == 0), stop=(j == 7))
                nc.tensor.matmul(out=sumsq_ps[:], lhsT=ones_r[:, 1:2], rhs=sq[:],
                                 start=(j == 0), stop=(j == 7))
            nmean = stp.tile([1, TC], f32, name="ln_nmean", tag="ln_nmean")
            nc.scalar.activation(out=nmean[:], in_=sum_ps[:], func=AF.Copy,
                                 scale=-1.0 / D)
            ms = stp.tile([1, TC], f32, name="ln_ms", tag="ln_ms")
            nc.scalar.activation(out=ms[:], in_=sumsq_ps[:], func=AF.Copy,
                                 scale=1.0 / D)
            m2 = stp.tile([1, TC], f32, name="ln_m2", tag="ln_m2")
            nc.vector.tensor_mul(out=m2[:], in0=nmean[:], in1=nmean[:])
            var = stp.tile([1, TC], f32, name="ln_var", tag="ln_var")
            nc.vector.tensor_tensor(out=var[:], in0=ms[:], in1=m2[:],
                                    op=ALU.subtract)
            std = stp.tile([1, TC], f32, name="ln_std", tag="ln_std")
            nc.scalar.activation(out=std[:], in_=var[:], func=AF.Sqrt,
                                 bias=eps_t[:], scale=1.0)
            rc = stp.tile([1, 2 * TC], f32r, name="ln_rc", tag="ln_rc")
            with nc.allow_low_precision(reason="f32r rounding of rstd intended"):
                nc.vector.reciprocal(out=rc[:, 0:TC], in_=std[:])
            nc.vector.tensor_mul(out=rc[:, TC:2 * TC], in0=nmean[:], in1=rc[:, 0:TC])
            bc_a = ps_bc.tile([P, TC], f32, name="bc_a", tag="bc_a")
            nc.tensor.matmul(out=bc_a[:], lhsT=ones_r[0:1, 0:P], rhs=rc[:, 0:TC],
                             start=True, stop=True)
            bc_c = ps_bc.tile([P, TC], f32, name="bc_c", tag="bc_c")
            nc.tensor.matmul(out=bc_c[:], lhsT=ones_r[0:1, 0:P],
                             rhs=rc[:, TC:2 * TC], start=True, stop=True)
            for j in range(8):
                t1 = sp.tile([P, TC], f32, name="ln_t1", tag="ln_t1")
                nc.vector.tensor_mul(out=t1[:], in0=xr[j][:], in1=bc_a[:])
                nc.vector.tensor_add(out=t1[:], in0=t1[:], in1=bc_c[:])
                nc.vector.tensor_scalar(
                    out=out_tiles[j][:], in0=t1[:],
                    scalar1=g_t[:, gcol + j:gcol + j + 1],
                    scalar2=b_t[:, bcol + j:bcol + j + 1],
                    op0=ALU.mult, op1=ALU.add)

        # ================= layers =================
        for l in range(L):
            lnp = sp.tile([P, 48], f32, name="lnp", tag="lnp")
            nc.sync.dma_start(out=lnp[:, 0:8], in_=ln1g[l].rearrange("(j p) -> p j", p=P))
            nc.sync.dma_start(out=lnp[:, 8:16], in_=ln1b[l].rearrange("(j p) -> p j", p=P))
            nc.sync.dma_start(out=lnp[:, 16:24], in_=ln2g[l].rearrange("(j p) -> p j", p=P))
            nc.sync.dma_start(out=lnp[:, 24:32], in_=ln2b[l].rearrange("(j p) -> p j", p=P))
            nc.sync.dma_start(out=lnp[:, 32:40], in_=bo_p[l].rearrange("(j p) -> p j", p=P))
            nc.sync.dma_start(out=lnp[:, 40:48], in_=b2_p[l].rearrange("(j p) -> p j", p=P))
            b1f_t = sp.tile([P, 32], f32, name="b1f_t", tag="b1f_t")
            nc.sync.dma_start(out=b1f_t[:], in_=b1_p[l].rearrange("(j p) -> p j", p=P))

            xln = xln_tiles()
            layer_norm(lnp, 0, lnp, 8, xln)

            # ---- K projection -> k_loc ----
            wk_l = wkT[l].rearrange("(k p) n -> p k n", p=P)
            for oc in range(8):
                wt = wA.tile([P, 8, P], f32r, name="wk_t", tag="wA")
                nc.sync.dma_start(out=wt[:], in_=wk_l[:, :, oc * P:(oc + 1) * P])
                ps = mmtile()
                for k in range(8):
                    nc.tensor.matmul(out=ps[:], lhsT=wt[:, k, :], rhs=xln[k][:],
                                     start=(k == 0), stop=(k == 7))
                ksb = kvp.tile([P, TC], f32r, name="ksb", tag="ksb")
                nc.scalar.activation(out=ksb[:], in_=ps[:], func=AF.Copy)
                nc.sync.dma_start(out=k_loc[oc * P:(oc + 1) * P, :], in_=ksb[:])

            # ---- V projection -> v_loc (head-quarter rows, 65-strided) ----
            wv_l = wvT[l].rearrange("(k p) n -> p k n", p=P)
            for hq in range(4):
                wt = wB.tile([P, 8, 256], f32r, name="wv_t", tag="wB")
                nc.sync.dma_start(out=wt[:], in_=wv_l[:, :, hq * 256:(hq + 1) * 256])
                for t4 in range(4):
                    ps = mmtile()
                    for k in range(8):
                        nc.tensor.matmul(out=ps[:, 0:256],
                                         lhsT=xln[k][:, t4 * P:(t4 + 1) * P],
                                         rhs=wt[:, k, :],
                                         start=(k == 0), stop=(k == 7))
                    vsb = kvp.tile([P, 256], f32r, name="vsb", tag="vsb")
                    nc.scalar.activation(out=vsb[:], in_=ps[:, 0:256], func=AF.Copy)
                    nc.sync.dma_start(
                        out=vl_view[hq, t4 * P:(t4 + 1) * P, 0:4, 0:64],
                        in_=vsb[:].rearrange("p (h c) -> p h c", c=64))

            k_ag, v_ag = k_ags[l], v_ags[l]
            if SKIP_COLL:
                nc.sync.dma_start(out=k_ag[0:D, :], in_=k_loc[:, :])
                nc.sync.dma_start(out=v_ag[0:4 * TC, :], in_=v_loc[:, :])
            else:
                nc.gpsimd.collective_compute(
                    "AllGather", ALU.bypass, replica_groups=RG,
                    ins=[k_loc.opt()], outs=[k_ag.opt()])
                nc.gpsimd.collective_compute(
                    "AllGather", ALU.bypass, replica_groups=RG,
                    ins=[v_loc.opt()], outs=[v_ag.opt()])

            # ---- attention, one head-quarter (4 heads) at a time ----
            o_pair = [big_tile(8 + p_, f"opair_{p_}") for p_ in range(8)]
            wq_l = wqT[l].rearrange("(k p) n -> p k n", p=P)
            for hq in range(4):
                # Q projection for heads 4hq..4hq+3, duplicated into both halves
                qh = [big_tile(hh, f"qh_{hh}") for hh in range(4)]
                for oci in range(2):
                    oc = 2 * hq + oci
                    wt = wA.tile([P, 8, P], f32r, name="wq_t", tag="wA")
                    nc.sync.dma_start(out=wt[:], in_=wq_l[:, :, oc * P:(oc + 1) * P])
                    ps = mmtile()
                    for k in range(8):
                        nc.tensor.matmul(out=ps[:], lhsT=wt[:, k, :], rhs=xln[k][:],
                                         start=(k == 0), stop=(k == 7))
                    he, ho = qh[2 * oci], qh[2 * oci + 1]
                    nc.scalar.activation(out=he[0:64, :], in_=ps[0:64, :], func=AF.Copy)
                    nc.scalar.activation(out=ho[64:P, :], in_=ps[64:P, :], func=AF.Copy)
                    nc.sync.dma_start(out=he[64:P, :], in_=he[0:64, :])
                    nc.sync.dma_start(out=ho[0:64, :], in_=ho[64:P, :])
                vts = []
                for kk in range(8):
                    vt = vtp.tile([P, 260], f32r, name=f"vt_{kk}", tag=f"vt_{kk}")
                    nc.gpsimd.indirect_dma_start(
                        out=vt[:], out_offset=None, in_=v_ag[:, :],
                        in_offset=bass.IndirectOffsetOnAxis(
                            ap=vidx_t[:, hq * 8 + kk:hq * 8 + kk + 1], axis=0))
                    vts.append(vt)
                for hh in range(4):
                    h = 4 * hq + hh
                    kt = ktp.tile([P, TC], f32r, name="kt", tag="kt")
                    nc.gpsimd.indirect_dma_start(
                        out=kt[:], out_offset=None, in_=k_ag[:, :],
                        in_offset=bass.IndirectOffsetOnAxis(
                            ap=kidx_t[:, h:h + 1], axis=0))
                    ops = ps_o.tile([65, TC], f32, name="ops", tag="ops")
                    for kk in range(8):
                        lo = kk < 4
                        j = kk % 4
                        qs0 = 0 if lo else QS[j]
                        nq = TC - qs0
                        base = 0 if lo else 64
                        sps = mmtile()
                        nc.tensor.matmul(
                            out=sps[:, 0:nq],
                            lhsT=kt[base:base + 64, j * P:(j + 1) * P],
                            rhs=qh[hh][base:base + 64, qs0:TC],
                            start=True, stop=True)
                        mt = m0_t[j] if lo else m1_t[j]
                        nc.vector.tensor_add(out=sps[:, 0:nq], in0=sps[:, 0:nq],
                                             in1=mt[:, 0:nq])
                        es = esp.tile([P, TC], f32r, name="es", tag="es")
                        nc.scalar.activation(out=es[:, 0:nq], in_=sps[:, 0:nq],
                                             func=AF.Exp, scale=HS ** -0.5)
                        nc.tensor.matmul(
                            out=ops[:, qs0:TC],
                            lhsT=vts[kk][:, 65 * hh:65 * hh + 65],
                            rhs=es[:, 0:nq],
                            start=(kk == 0), stop=(kk == 7))
                    recip = otp.tile([65, TC], f32r, name="recip", tag="recip")
                    with nc.allow_low_precision(reason="f32r softmax denom"):
                        nc.vector.reciprocal(out=recip[64:65, :], in_=ops[64:65, :])
                    bc = ps_bc.tile([P, TC], f32, name="bc_o", tag="bc_a")
                    nc.tensor.matmul(out=bc[0:64, :], lhsT=ones_r[64:65, 0:64],
                                     rhs=recip[64:65, :], start=True, stop=True)
                    bcs = otp.tile([64, TC], f32r, name="bcs", tag="bcs")
                    nc.scalar.activation(out=bcs[:], in_=bc[0:64, :], func=AF.Copy)
                    o_t = otp.tile([64, TC], f32r, name="o_t", tag="o_t")
                    nc.vector.tensor_mul(out=o_t[:], in0=ops[0:64, :], in1=bcs[:])
                    nc.sync.dma_start(
                        out=o_pair[h // 2][64 * (h % 2):64 * (h % 2) + 64, :],
                        in_=o_t[:])

            # ---- attention out projection + residual ----
            wo_l = wo[l].rearrange("(k p) n -> p k n", p=P)
            for dout in range(8):
                wt = wA.tile([P, 8, P], f32r, name="wo_t", tag="wA")
                nc.sync.dma_start(out=wt[:], in_=wo_l[:, :, dout * P:(dout + 1) * P])
                ps = mmtile()
                for k in range(8):
                    nc.tensor.matmul(out=ps[:], lhsT=wt[:, k, :], rhs=o_pair[k][:],
                                     start=(k == 0), stop=(k == 7))
                ysb = sp.tile([P, TC], f32, name="ysb", tag="ysb")
                nc.scalar.activation(out=ysb[:], in_=ps[:], func=AF.Identity,
                                     bias=lnp[:, 32 + dout:33 + dout], scale=1.0)
                nc.vector.tensor_add(out=xr[dout][:], in0=xr[dout][:], in1=ysb[:])

            # ---- FFN ----
            xln2 = xln_tiles()
            layer_norm(lnp, 16, lnp, 24, xln2)

            w1_l = w1[l].rearrange("(k p) n -> p k n", p=P)
            w2_l = w2[l].rearrange("(k p) n -> p k n", p=P)
            y2 = [big_tile(8 + d_, f"y2_{d_}", dtype=f32) for d_ in range(8)]
            for blk in range(4):
                h1 = [big_tile(c_, f"h1_{c_}") for c_ in range(8)]
                for ci in range(8):
                    hc = 8 * blk + ci
                    wt = wA.tile([P, 8, P], f32r, name="w1_t", tag="wA")
                    nc.sync.dma_start(out=wt[:], in_=w1_l[:, :, hc * P:(hc + 1) * P])
                    ps = mmtile()
                    for k in range(8):
                        nc.tensor.matmul(out=ps[:], lhsT=wt[:, k, :], rhs=xln2[k][:],
                                         start=(k == 0), stop=(k == 7))
                    nc.scalar.activation(out=h1[ci][:], in_=ps[:], func=AF.Relu,
                                         bias=b1f_t[:, hc:hc + 1], scale=1.0)
                for dout in range(8):
                    wt = wA.tile([P, 8, P], f32r, name="w2_t", tag="wA")
                    nc.sync.dma_start(
                        out=wt[:],
                        in_=w2_l[:, 8 * blk:8 * blk + 8, dout * P:(dout + 1) * P])
                    ps = mmtile()
                    for c in range(8):
                        nc.tensor.matmul(out=ps[:], lhsT=wt[:, c, :], rhs=h1[c][:],
                                         start=(c == 0), stop=(c == 7))
                    if blk == 0:
                        nc.scalar.activation(out=y2[dout][:], in_=ps[:],
                                             func=AF.Identity,
                                             bias=lnp[:, 40 + dout:41 + dout],
                                             scale=1.0)
                    else:
                        nc.vector.tensor_add(out=y2[dout][:], in0=y2[dout][:],
                                             in1=ps[:])
            for dout in range(8):
                nc.vector.tensor_add(out=xr[dout][:], in0=xr[dout][:],
                                     in1=y2[dout][:])

        # ---- final LN -> xf_loc -> AllGather ----
        lnf_t = sp.tile([P, 16], f32, name="lnf_t", tag="lnp")
        nc.sync.dma_start(out=lnf_t[:, 0:8],
                          in_=lnfg.ap().rearrange("o (j p) -> p (o j)", p=P))
        nc.sync.dma_start(out=lnf_t[:, 8:16],
                          in_=lnfb.ap().rearrange("o (j p) -> p (o j)", p=P))
        xlnf = xln_tiles()
        layer_norm(lnf_t, 0, lnf_t, 8, xlnf)
        for j in range(8):
            nc.sync.dma_start(out=xf_loc[j * P:(j + 1) * P, :], in_=xlnf[j][:])
        if SKIP_COLL:
            nc.sync.dma_start(out=xf_ag[0:D, :], in_=xf_loc[:, :])
        else:
            nc.gpsimd.collective_compute(
                "AllGather", ALU.bypass, replica_groups=RG,
                ins=[xf_loc.opt()], outs=[xf_ag.opt()])

        est.close()

        # ================= LM head (vocab-sharded) =================
        if SKIP_LM:
            outer.close()
            nc.compile()
            return nc
        with tc.tile_pool(name="lmxf", bufs=1) as lmxf, \
             tc.tile_pool(name="lmw", bufs=2) as lmw, \
             tc.tile_pool(name="lmo", bufs=3) as lmo, \
             tc.tile_pool(name="lmb", bufs=2) as lmb, \
             tc.tile_pool(name="ps_lm", bufs=4, space="PSUM") as ps_lm:
            xf_view = xf_ag[:].rearrange("(r j p) w -> p j r w", p=P, j=8)
            xf_t = []
            for j in range(8):
                t = lmxf.tile([P, NCORES, TC], f32r, name=f"xf_{j}", tag=f"xf_{j}")
                nc.sync.dma_start(out=t[:], in_=xf_view[:, j])
                xf_t.append(t[:].rearrange("p r w -> p (r w)"))
            wout_v = wout.rearrange("(k p) n -> p k n", p=P)
            for vs in range(8):
                bt = lmb.tile([1, 500], f32r, name="bt", tag="bt")
                nc.sync.dma_start(out=bt[:], in_=bout[:, vs * 500:(vs + 1) * 500])
                bp = ps_lm.tile([P, TC], f32, name="lm_bps", tag="lm")
                nc.tensor.matmul(out=bp[:, 0:500], lhsT=ones_r[0:1, 0:P],
                                 rhs=bt[:], start=True, stop=True)
                bias_vs = lmb.tile([P, 500], f32, name="bias_vs", tag="bias_vs")
                nc.scalar.activation(out=bias_vs[:], in_=bp[:, 0:500], func=AF.Copy)
                wt = lmw.tile([P, 8, 500], f32r, name="wout_t", tag="wout")
                nc.sync.dma_start(out=wt[:], in_=wout_v[:, :, vs * 500:(vs + 1) * 500])
                for m in range(32):
                    ps = ps_lm.tile([P, TC], f32, name="lm_ps", tag="lm")
                    for k in range(8):
                        nc.tensor.matmul(out=ps[:, 0:500],
                                         lhsT=xf_t[k][:, m * P:(m + 1) * P],
                                         rhs=wt[:, k, :],
                                         start=(k == 0), stop=(k == 7))
                    osb = lmo.tile([P, TC], f32, name="osb", tag="osb")
                    nc.vector.tensor_add(out=osb[:, 0:500], in0=ps[:, 0:500],
                                         in1=bias_vs[:])
                    nc.sync.dma_start(
                        out=out[m * P:(m + 1) * P, vs * 500:(vs + 1) * 500],
                        in_=osb[:, 0:500])
        outer.close()

    nc.compile()
    return nc


def _prep_inputs(inputs):
    """Shard/reformat host inputs into 8 per-core input maps."""
    import ml_dtypes
    inp = {k: np.asarray(v) for k, v in inputs.items()}
    tok = inp['input_tokens'].astype(np.int32)          # [B, T]
    shared = {
        'tokemb': np.ascontiguousarray(inp['tok_emb'], dtype=np.float32),
        'wqT': np.ascontiguousarray(
            inp['Wq'].transpose(0, 2, 1, 3).reshape(L, D, H * HS), dtype=np.float32),
        'wkT': np.ascontiguousarray(
            inp['Wk'].transpose(0, 2, 1, 3).reshape(L, D, H * HS), dtype=np.float32),
        'wvT': np.ascontiguousarray(
            inp['Wv'].transpose(0, 2, 1, 3).reshape(L, D, H * HS), dtype=np.float32),
        'wo': np.ascontiguousarray(inp['Wo'], dtype=np.float32),
        'w1': np.ascontiguousarray(inp['W1'], dtype=np.float32),
        'w2': np.ascontiguousarray(inp['W2'], dtype=np.float32),
        'ln1g': np.ascontiguousarray(inp['ln1_g'], dtype=np.float32),
        'ln1b': np.ascontiguousarray(inp['ln1_b'], dtype=np.float32),
        'ln2g': np.ascontiguousarray(inp['ln2_g'], dtype=np.float32),
        'ln2b': np.ascontiguousarray(inp['ln2_b'], dtype=np.float32),
        'lnfg': np.ascontiguousarray(inp['lnf_g'].reshape(1, D), dtype=np.float32),
        'lnfb': np.ascontiguousarray(inp['lnf_b'].reshape(1, D), dtype=np.float32),
        'bo': np.ascontiguousarray(inp['bo'], dtype=np.float32),
        'b1': np.ascontiguousarray(inp['b1'], dtype=np.float32),
        'b2': np.ascontiguousarray(inp['b2'], dtype=np.float32),
    }
    pe = np.asarray(inp['pos_emb'], dtype=np.float32)
    wout_full = np.asarray(inp['W_out'], dtype=np.float32)
    bout_full = np.asarray(inp['b_out'], dtype=np.float32)

    pcol = np.arange(P)
    in_maps = []
    for c in range(NCORES):
        b, hf = c // 2, c % 2
        m = dict(shared)
        m['tokidx'] = np.ascontiguousarray(
            tok[b, hf * TC:(hf + 1) * TC].reshape(TC, 1))
        m['pos'] = np.ascontiguousarray(pe[hf * TC:(hf + 1) * TC])
        m['wout'] = np.ascontiguousarray(wout_full[:, c * VS:(c + 1) * VS])
        m['bout'] = np.ascontiguousarray(bout_full[c * VS:(c + 1) * VS].reshape(1, VS))
        # K gather indices into k_ag [8*1024, 512]
        kix = np.empty((P, H), np.int32)
        for h in range(H):
            r = 2 * b + (pcol >= 64)
            kix[:, h] = D * r + 64 * h + (pcol % 64)
        m['kidx'] = kix
        # V gather indices into v_ag [8*4*512, 260]
        vix = np.empty((P, 32), np.int32)
        for hq in range(4):
            for kk in range(8):
                r = 2 * b + (1 if kk >= 4 else 0)
                vix[:, hq * 8 + kk] = (4 * TC) * r + TC * hq + P * (kk % 4) + pcol
        m['vidx'] = vix
        # additive masks (bf16-exact values)
        m0 = np.zeros((4, P, TC), np.float32)
        m1 = np.zeros((4, P, TC), np.float32)
        if hf == 0:
            for j in range(4):
                key = P * j + pcol[:, None]
                q = np.arange(TC)[None, :]
                m0[j] = np.where(q >= key, 0.0, NEG)
            m1[:] = NEG
        else:
            for j in range(4):
                qs0 = QS[j]
                key_local = P * j + pcol[:, None]
                q_local = qs0 + np.arange(TC)[None, :]
                m1[j] = np.where(q_local >= key_local, 0.0, NEG)
        m['mask0'] = m0.astype(ml_dtypes.bfloat16)
        m['mask1'] = m1.astype(ml_dtypes.bfloat16)
        in_maps.append(m)
    return in_maps


def _enable_jax_cache():
    try:
        import jax
        jax.config.update("jax_compilation_cache_dir", "/tmp/jax_neff_cache")
        jax.config.update("jax_persistent_cache_min_compile_time_secs", 0.0)
        jax.config.update("jax_persistent_cache_min_entry_size_bytes", -1)
    except Exception:
        pass


def kernel(**inputs):
    global LAST_RESULTS
    _enable_jax_cache()
    from concourse.bass_utils import run_bass_kernel_spmd
    if 'nc' not in _CACHE:
        _CACHE['nc'] = _build()
    nc = _CACHE['nc']
    in_maps = _prep_inputs(inputs)
    kw = {}
    if TRACE:
        kw = dict(trace=True, trace_cores=list(range(NCORES)), stitch_traces=False)
    res = run_bass_kernel_spmd(nc, in_maps, core_ids=list(range(NCORES)), **kw)
    LAST_RESULTS = res
    full = np.concatenate([res.results[c]['out'] for c in range(NCORES)], axis=1)
    return np.ascontiguousarray(full.reshape(B, T, V), dtype=np.float32)

